# revision 5
# baseline (speedup 1.0000x reference)
"""MoE MLP (top-k == E) Trainium2 Bass kernel, expert-parallel over 8 NeuronCores.

Reference computation (per token x, experts e = 0..7):
    logits = x @ Wr.T                       (N, E)
    probs  = softmax(logits)                (N, E)
    w      = sort_desc(probs)               (N, E)   # rank-sorted, NOT matched to expert id
    hmid_e = gelu_erf(x @ W1[e].T)          (N, I)
    y_e    = hmid_e @ W2[e].T               (N, H)
    out    = sum_e w[:, e] * y_e            (N, H)

Sharding: core e owns expert e (W1[e], W2[e]); tokens + router replicated.
Each core computes w[:, e] (the e-th largest softmax prob, selected via a
one-hot per-core input) and its weighted y_e partial; a per-token-block
ReduceScatter sums partials over cores, leaving each core with a 1/8 shard
of every block; the host concatenates shards.

Device data layouts (prepared host-side in kernel()):
    xt  (H, N)            x transposed (contraction dim H on partitions)
    wr  (128, KH*E)       wr[p, k*E+e] = Wr[e, k*128+p]
    w1  (IC, 128, H)      w1[ic, p, k*128+m] = W1[e][ic*128+m, k*128+p]  (= W1[e].T blocked)
    w2  (IC, 128, H)      w2[ic, p, h] = W2[e][h, ic*128+p]              (= W2[e].T strips)
    sel (128, E)          one-hot row (replicated over partitions) selecting rank e
    out (NBLK, NB//8, H)  per-core output: RS shard of every token block
"""

import numpy as np

import concourse.bacc as bacc
import concourse.bass as bass
import concourse.mybir as mybir
import concourse.tile as tile
from concourse.bass_utils import run_bass_kernel_spmd

AF = mybir.ActivationFunctionType
ALU = mybir.AluOpType
F32 = mybir.dt.float32

N_CORES = 8
B, S, H, I, E = 2, 2048, 1024, 4096, 8
N = B * S              # 4096 tokens
NB = 512               # tokens per block
NBLK = N // NB         # 8 blocks
TPB = NB // 128        # 4 token tiles per block
KH = H // 128          # 8 contraction chunks for MM1
IC = I // 128          # 32 contraction chunks for MM2 / output chunks of MM1
HO = H // 512          # 2 moving-free slices of H in MM2


def build_nc(dt_mm=F32):
    nc = bacc.Bacc(None, num_devices=N_CORES)

    xt = nc.declare_dram_parameter("xt", [H, N], dt_mm, isOutput=False)
    wr = nc.declare_dram_parameter("wr", [128, KH * E], dt_mm, isOutput=False)
    w1 = nc.declare_dram_parameter("w1", [IC, 128, H], dt_mm, isOutput=False)
    w2 = nc.declare_dram_parameter("w2", [IC, 128, H], dt_mm, isOutput=False)
    sel = nc.declare_dram_parameter("sel", [128, E], F32, isOutput=False)
    out = nc.declare_dram_parameter("out", [NBLK, NB // N_CORES, H], F32, isOutput=True)

    xt_v = xt.ap().rearrange("(k p) n -> p k n", p=128)  # [p][k][n]

    with tile.TileContext(nc) as tc:
        with (
            tc.tile_pool(name="weights", bufs=1) as wpool,
            tc.tile_pool(name="io", bufs=2) as iopool,
            tc.tile_pool(name="stream", bufs=3) as spool,
            tc.tile_pool(name="hmidp", bufs=1) as hpool,
            tc.tile_pool(name="small", bufs=2) as smpool,
            tc.tile_pool(name="psum", bufs=1, space="PSUM") as pspool,
            tc.tile_pool(name="dram", bufs=2, space="DRAM") as dpool,
        ):
            wr_sb = wpool.tile([128, KH * E], dt_mm, name="wr_sb", tag="wr_sb")
            nc.sync.dma_start(wr_sb[:], wr[:, :])
            sel_sb = wpool.tile([128, E], F32, name="sel_sb", tag="sel_sb")
            nc.sync.dma_start(sel_sb[:], sel[:, :])

            for b in range(NBLK):
                # ---- load xT block: (128, KH, NB)
                xtb = iopool.tile([128, KH, NB], dt_mm, name=f"xtb{b}", tag="xtb")
                nc.sync.dma_start(xtb[:], xt_v[:, :, b * NB:(b + 1) * NB])

                # ---- routing for the block's TPB token tiles
                wsel = smpool.tile([128, TPB], F32, name=f"wsel{b}", tag="wsel")
                for t in range(TPB):
                    tok = slice(t * 128, (t + 1) * 128)
                    lg = pspool.tile([128, E], F32, name=f"lg{b}_{t}", tag="lg", bufs=2)
                    for k in range(KH):
                        nc.tensor.matmul(
                            lg[:],
                            xtb[:, k, tok],
                            wr_sb[:, k * E:(k + 1) * E],
                            start=(k == 0),
                            stop=(k == KH - 1),
                        )
                    negm = smpool.tile([128, 1], F32, name=f"negm{b}_{t}", tag="negm")
                    nc.vector.tensor_reduce(negm[:], lg[:], axis=mybir.AxisListType.X,
                                            op=ALU.max, negate=True)
                    ev = smpool.tile([128, E], F32, name=f"ev{b}_{t}", tag="ev")
                    sumexp = smpool.tile([128, 1], F32, name=f"sumexp{b}_{t}", tag="sumexp")
                    nc.scalar.activation(ev[:], lg[:], AF.Exp, bias=negm[:],
                                         accum_out=sumexp[:])
                    rsum = smpool.tile([128, 1], F32, name=f"rsum{b}_{t}", tag="rsum")
                    nc.vector.reciprocal(rsum[:], sumexp[:])
                    # extract ranks 0..E-1 (descending) via max + mask-to-zero
                    wv = smpool.tile([128, E], F32, name=f"wv{b}_{t}", tag="wv")
                    for r in range(E):
                        nc.vector.tensor_reduce(wv[:, r:r + 1], ev[:],
                                                axis=mybir.AxisListType.X, op=ALU.max)
                        if r < E - 1:
                            # ev = (ev < max) * ev  : zero out the current max
                            nc.vector.scalar_tensor_tensor(
                                ev[:], ev[:], wv[:, r:r + 1], ev[:],
                                op0=ALU.is_lt, op1=ALU.mult)
                    # select this core's rank via one-hot, normalize by 1/sumexp
                    tmp8 = smpool.tile([128, E], F32, name=f"tmp8{b}_{t}", tag="tmp8")
                    nc.vector.tensor_mul(tmp8[:], wv[:], sel_sb[:])
                    nc.vector.tensor_reduce(wsel[:, t:t + 1], tmp8[:],
                                            axis=mybir.AxisListType.X, op=ALU.add)
                    nc.vector.tensor_mul(wsel[:, t:t + 1], wsel[:, t:t + 1], rsum[:])

                # ---- MM1 + GELU: hmid[ic] = gelu(W1_ic @ x_blk^T)  (128, NB) each
                hmid = hpool.tile([128, IC, NB], dt_mm, name=f"hmid{b}", tag="hmid")
                for ic in range(IC):
                    w1t = spool.tile([128, H], dt_mm, name=f"w1t{b}_{ic}", tag="w1t")
                    nc.sync.dma_start(w1t[:], w1[ic])
                    ps1 = pspool.tile([128, NB], F32, name=f"ps1{b}_{ic}", tag="ps1",
                                      bufs=2)
                    for k in range(KH):
                        nc.tensor.matmul(
                            ps1[:],
                            w1t[:, k * 128:(k + 1) * 128],
                            xtb[:, k, :],
                            start=(k == 0),
                            stop=(k == KH - 1),
                        )
                    nc.scalar.activation(hmid[:, ic, :], ps1[:], AF.Gelu)

                # ---- MM2: out[t] += hmid^T @ W2^T, weighted by wsel
                outsb = [
                    iopool.tile([128, H], F32, name=f"outsb{b}_{t}", tag=f"outsb{t}")
                    for t in range(TPB)
                ]
                for h in range(HO):
                    hsl = slice(h * 512, (h + 1) * 512)
                    ps2 = [
                        pspool.tile([128, 512], F32, name=f"ps2_{b}_{h}_{t}",
                                    tag=f"ps2_{t}", bufs=1)
                        for t in range(TPB)
                    ]
                    for ic in range(IC):
                        w2t = spool.tile([128, 512], dt_mm, name=f"w2t{b}_{h}_{ic}",
                                         tag="w2t")
                        nc.sync.dma_start(w2t[:], w2[ic, :, hsl])
                        for t in range(TPB):
                            nc.tensor.matmul(
                                ps2[t][:],
                                hmid[:, ic, t * 128:(t + 1) * 128],
                                w2t[:],
                                start=(ic == 0),
                                stop=(ic == IC - 1),
                            )
                    for t in range(TPB):
                        nc.vector.tensor_scalar_mul(outsb[t][:, hsl], ps2[t][:],
                                                    wsel[:, t:t + 1])

                # ---- combine over cores: ReduceScatter this block's partial
                rs_in = dpool.tile([NB, H], F32, name=f"rsin{b}", tag="rsin")
                for t in range(TPB):
                    nc.sync.dma_start(rs_in[t * 128:(t + 1) * 128, :], outsb[t][:])
                rs_out = dpool.tile([NB // N_CORES, H], F32, name=f"rsout{b}",
                                    tag="rsout")
                nc.gpsimd.collective_compute(
                    "ReduceScatter",
                    ALU.add,
                    replica_groups=[list(range(N_CORES))],
                    ins=[rs_in.opt()],
                    outs=[rs_out.opt()],
                )
                nc.sync.dma_start(out[b], rs_out[:])

    nc.compile()
    return nc


def _prep_inputs(hidden_states, Wr, W1, W2, np_dt=np.float32):
    x = np.ascontiguousarray(
        np.asarray(hidden_states, dtype=np.float32).reshape(N, H))
    xT = np.ascontiguousarray(x.T).astype(np_dt, copy=False)
    Wr = np.asarray(Wr, dtype=np.float32)
    W1 = np.asarray(W1, dtype=np.float32)
    W2 = np.asarray(W2, dtype=np.float32)
    wrb = np.ascontiguousarray(
        Wr.T.reshape(KH, 128, E).transpose(1, 0, 2)).reshape(128, KH * E)
    wrb = wrb.astype(np_dt, copy=False)
    in_maps = []
    for e in range(N_CORES):
        w1b = np.ascontiguousarray(
            W1[e].reshape(IC, 128, KH, 128).transpose(0, 3, 2, 1)
        ).reshape(IC, 128, H).astype(np_dt, copy=False)
        w2b = np.ascontiguousarray(W2[e].T).reshape(IC, 128, H).astype(
            np_dt, copy=False)
        sel = np.zeros((128, E), np.float32)
        sel[:, e] = 1.0
        in_maps.append({"xt": xT, "wr": wrb, "w1": w1b, "w2": w2b, "sel": sel})
    return in_maps


_CACHE = {}


def run(inputs, trace=False, dt_mm=F32, np_dt=np.float32):
    key = str(dt_mm)
    if key not in _CACHE:
        _CACHE[key] = build_nc(dt_mm)
    nc = _CACHE[key]
    in_maps = _prep_inputs(inputs["hidden_states"], inputs["Wr"], inputs["W1"],
                           inputs["W2"], np_dt=np_dt)
    res = run_bass_kernel_spmd(nc, in_maps, list(range(N_CORES)), trace=trace)
    shards = np.stack([np.asarray(res.results[r]["out"]) for r in range(N_CORES)],
                      axis=1)  # (NBLK, cores, NB//cores, H)
    full = shards.reshape(B, S, H).astype(np.float32, copy=False)
    return full, res


def kernel(**inputs) -> np.ndarray:
    full, _ = run(inputs, trace=False)
    return full


# revision 6
# speedup vs baseline: 1.0133x; 1.0133x over previous
"""MoE MLP (top-k == E) Trainium2 Bass kernel, expert-parallel over 8 NeuronCores.

Reference computation (per token x, experts e = 0..7):
    logits = x @ Wr.T                       (N, E)
    probs  = softmax(logits)                (N, E)
    w      = sort_desc(probs)               (N, E)   # rank-sorted, NOT matched to expert id
    hmid_e = gelu_erf(x @ W1[e].T)          (N, I)
    y_e    = hmid_e @ W2[e].T               (N, H)
    out    = sum_e w[:, e] * y_e            (N, H)

Sharding: core e owns expert e (W1[e], W2[e]); tokens + router replicated.
Each core computes w[:, e] (the e-th largest softmax prob, selected via a
one-hot per-core input) and its weighted y_e partial; a per-token-block
ReduceScatter sums partials over cores, leaving each core with a 1/8 shard
of every block; the host concatenates shards.

Device data layouts (prepared host-side in kernel()):
    xt  (H, N)            x transposed (contraction dim H on partitions)
    wr  (128, KH*E)       wr[p, k*E+e] = Wr[e, k*128+p]
    w1  (IC, 128, H)      w1[ic, p, k*128+m] = W1[e][ic*128+m, k*128+p]  (= W1[e].T blocked)
    w2  (IC, 128, H)      w2[ic, p, h] = W2[e][h, ic*128+p]              (= W2[e].T strips)
    sel (128, E)          one-hot row (replicated over partitions) selecting rank e
    out (NBLK, NB//8, H)  per-core output: RS shard of every token block
"""

import numpy as np

import concourse.bacc as bacc
import concourse.bass as bass
import concourse.mybir as mybir
import concourse.tile as tile
from concourse.bass_utils import run_bass_kernel_spmd

AF = mybir.ActivationFunctionType
ALU = mybir.AluOpType
F32 = mybir.dt.float32

N_CORES = 8
B, S, H, I, E = 2, 2048, 1024, 4096, 8
N = B * S              # 4096 tokens
NB = 512               # tokens per block
NBLK = N // NB         # 8 blocks
TPB = NB // 128        # 4 token tiles per block
KH = H // 128          # 8 contraction chunks for MM1
IC = I // 128          # 32 contraction chunks for MM2 / output chunks of MM1
HO = H // 512          # 2 moving-free slices of H in MM2


def build_nc(dt_mm=F32):
    nc = bacc.Bacc(None, num_devices=N_CORES)

    xt = nc.declare_dram_parameter("xt", [H, N], dt_mm, isOutput=False)
    wr = nc.declare_dram_parameter("wr", [128, KH * E], dt_mm, isOutput=False)
    w1 = nc.declare_dram_parameter("w1", [IC, 128, H], dt_mm, isOutput=False)
    w2 = nc.declare_dram_parameter("w2", [IC, 128, H], dt_mm, isOutput=False)
    sel = nc.declare_dram_parameter("sel", [128, E], F32, isOutput=False)
    out = nc.declare_dram_parameter("out", [NBLK, NB // N_CORES, H], F32, isOutput=True)

    xt_v = xt.ap().rearrange("(k p) n -> p k n", p=128)  # [p][k][n]

    with tile.TileContext(nc) as tc:
        with (
            tc.tile_pool(name="weights", bufs=1) as wpool,
            tc.tile_pool(name="io", bufs=3) as iopool,
            tc.tile_pool(name="stream", bufs=6) as spool,
            tc.tile_pool(name="hmidp", bufs=1) as hpool,
            tc.tile_pool(name="small", bufs=2) as smpool,
            tc.tile_pool(name="psum", bufs=1, space="PSUM") as pspool,
            tc.tile_pool(name="dram", bufs=2, space="DRAM") as dpool,
        ):
            wr_sb = wpool.tile([128, KH * E], dt_mm, name="wr_sb", tag="wr_sb")
            nc.sync.dma_start(wr_sb[:], wr[:, :])
            sel_sb = wpool.tile([128, E], F32, name="sel_sb", tag="sel_sb")
            nc.sync.dma_start(sel_sb[:], sel[:, :])

            for b in range(NBLK):
                # ---- load xT block: (128, KH, NB)
                xtb = iopool.tile([128, KH, NB], dt_mm, name=f"xtb{b}", tag="xtb")
                nc.sync.dma_start(xtb[:], xt_v[:, :, b * NB:(b + 1) * NB])

                # ---- routing for the block's TPB token tiles
                wsel = smpool.tile([128, TPB], F32, name=f"wsel{b}", tag="wsel")
                for t in range(TPB):
                    tok = slice(t * 128, (t + 1) * 128)
                    lg = pspool.tile([128, E], F32, name=f"lg{b}_{t}", tag="lg", bufs=1)
                    for k in range(KH):
                        nc.tensor.matmul(
                            lg[:],
                            xtb[:, k, tok],
                            wr_sb[:, k * E:(k + 1) * E],
                            start=(k == 0),
                            stop=(k == KH - 1),
                        )
                    negm = smpool.tile([128, 1], F32, name=f"negm{b}_{t}", tag="negm")
                    nc.vector.tensor_reduce(negm[:], lg[:], axis=mybir.AxisListType.X,
                                            op=ALU.max, negate=True)
                    ev = smpool.tile([128, E], F32, name=f"ev{b}_{t}", tag="ev")
                    sumexp = smpool.tile([128, 1], F32, name=f"sumexp{b}_{t}", tag="sumexp")
                    nc.scalar.activation(ev[:], lg[:], AF.Exp, bias=negm[:],
                                         accum_out=sumexp[:])
                    rsum = smpool.tile([128, 1], F32, name=f"rsum{b}_{t}", tag="rsum")
                    nc.vector.reciprocal(rsum[:], sumexp[:])
                    # extract ranks 0..E-1 (descending) via max + mask-to-zero
                    wv = smpool.tile([128, E], F32, name=f"wv{b}_{t}", tag="wv")
                    for r in range(E):
                        nc.vector.tensor_reduce(wv[:, r:r + 1], ev[:],
                                                axis=mybir.AxisListType.X, op=ALU.max)
                        if r < E - 1:
                            # ev = (ev < max) * ev  : zero out the current max
                            nc.vector.scalar_tensor_tensor(
                                ev[:], ev[:], wv[:, r:r + 1], ev[:],
                                op0=ALU.is_lt, op1=ALU.mult)
                    # select this core's rank via one-hot, normalize by 1/sumexp
                    tmp8 = smpool.tile([128, E], F32, name=f"tmp8{b}_{t}", tag="tmp8")
                    nc.vector.tensor_mul(tmp8[:], wv[:], sel_sb[:])
                    nc.vector.tensor_reduce(wsel[:, t:t + 1], tmp8[:],
                                            axis=mybir.AxisListType.X, op=ALU.add)
                    nc.vector.tensor_mul(wsel[:, t:t + 1], wsel[:, t:t + 1], rsum[:])

                # ---- MM1 + GELU: hmid[ic] = gelu(W1_ic @ x_blk^T)  (128, NB) each
                hmid = hpool.tile([128, IC, NB], dt_mm, name=f"hmid{b}", tag="hmid")
                for ic in range(IC):
                    w1t = spool.tile([128, H], dt_mm, name=f"w1t{b}_{ic}", tag="w1t")
                    nc.sync.dma_start(w1t[:], w1[ic])
                    ps1 = pspool.tile([128, NB], F32, name=f"ps1{b}_{ic}", tag="ps1",
                                      bufs=3)
                    for k in range(KH):
                        nc.tensor.matmul(
                            ps1[:],
                            w1t[:, k * 128:(k + 1) * 128],
                            xtb[:, k, :],
                            start=(k == 0),
                            stop=(k == KH - 1),
                        )
                    nc.scalar.activation(hmid[:, ic, :], ps1[:], AF.Gelu)

                # ---- MM2: out[t] += hmid^T @ W2^T, weighted by wsel
                outsb = [
                    iopool.tile([128, H], F32, name=f"outsb{b}_{t}", tag=f"outsb{t}")
                    for t in range(TPB)
                ]
                for h in range(HO):
                    hsl = slice(h * 512, (h + 1) * 512)
                    ps2 = [
                        pspool.tile([128, 512], F32, name=f"ps2_{b}_{h}_{t}",
                                    tag=f"ps2_{t}", bufs=1)
                        for t in range(TPB)
                    ]
                    for ic in range(IC):
                        w2t = spool.tile([128, 512], dt_mm, name=f"w2t{b}_{h}_{ic}",
                                         tag="w2t")
                        nc.sync.dma_start(w2t[:], w2[ic, :, hsl])
                        for t in range(TPB):
                            nc.tensor.matmul(
                                ps2[t][:],
                                hmid[:, ic, t * 128:(t + 1) * 128],
                                w2t[:],
                                start=(ic == 0),
                                stop=(ic == IC - 1),
                            )
                    for t in range(TPB):
                        nc.vector.tensor_scalar_mul(outsb[t][:, hsl], ps2[t][:],
                                                    wsel[:, t:t + 1])

                # ---- combine over cores: ReduceScatter this block's partial
                rs_in = dpool.tile([NB, H], F32, name=f"rsin{b}", tag="rsin")
                for t in range(TPB):
                    nc.sync.dma_start(rs_in[t * 128:(t + 1) * 128, :], outsb[t][:])
                rs_out = dpool.tile([NB // N_CORES, H], F32, name=f"rsout{b}",
                                    tag="rsout")
                nc.gpsimd.collective_compute(
                    "ReduceScatter",
                    ALU.add,
                    replica_groups=[list(range(N_CORES))],
                    ins=[rs_in.opt()],
                    outs=[rs_out.opt()],
                )
                nc.sync.dma_start(out[b], rs_out[:])

    nc.compile()
    return nc


def _prep_inputs(hidden_states, Wr, W1, W2, np_dt=np.float32):
    x = np.ascontiguousarray(
        np.asarray(hidden_states, dtype=np.float32).reshape(N, H))
    xT = np.ascontiguousarray(x.T).astype(np_dt, copy=False)
    Wr = np.asarray(Wr, dtype=np.float32)
    W1 = np.asarray(W1, dtype=np.float32)
    W2 = np.asarray(W2, dtype=np.float32)
    wrb = np.ascontiguousarray(
        Wr.T.reshape(KH, 128, E).transpose(1, 0, 2)).reshape(128, KH * E)
    wrb = wrb.astype(np_dt, copy=False)
    in_maps = []
    for e in range(N_CORES):
        w1b = np.ascontiguousarray(
            W1[e].reshape(IC, 128, KH, 128).transpose(0, 3, 2, 1)
        ).reshape(IC, 128, H).astype(np_dt, copy=False)
        w2b = np.ascontiguousarray(W2[e].T).reshape(IC, 128, H).astype(
            np_dt, copy=False)
        sel = np.zeros((128, E), np.float32)
        sel[:, e] = 1.0
        in_maps.append({"xt": xT, "wr": wrb, "w1": w1b, "w2": w2b, "sel": sel})
    return in_maps


_CACHE = {}


def run(inputs, trace=False, dt_mm=F32, np_dt=np.float32):
    key = str(dt_mm)
    if key not in _CACHE:
        _CACHE[key] = build_nc(dt_mm)
    nc = _CACHE[key]
    in_maps = _prep_inputs(inputs["hidden_states"], inputs["Wr"], inputs["W1"],
                           inputs["W2"], np_dt=np_dt)
    res = run_bass_kernel_spmd(nc, in_maps, list(range(N_CORES)), trace=trace)
    shards = np.stack([np.asarray(res.results[r]["out"]) for r in range(N_CORES)],
                      axis=1)  # (NBLK, cores, NB//cores, H)
    full = shards.reshape(B, S, H).astype(np.float32, copy=False)
    return full, res


def kernel(**inputs) -> np.ndarray:
    full, _ = run(inputs, trace=False)
    return full


# revision 7
# speedup vs baseline: 1.0515x; 1.0377x over previous
"""MoE MLP (top-k == E) Trainium2 Bass kernel, expert-parallel over 8 NeuronCores.

Reference computation (per token x, experts e = 0..7):
    logits = x @ Wr.T                       (N, E)
    probs  = softmax(logits)                (N, E)
    w      = sort_desc(probs)               (N, E)   # rank-sorted, NOT matched to expert id
    hmid_e = gelu_erf(x @ W1[e].T)          (N, I)
    y_e    = hmid_e @ W2[e].T               (N, H)
    out    = sum_e w[:, e] * y_e            (N, H)

Sharding: core e owns expert e (W1[e], W2[e]); tokens + router replicated.
Each core computes w[:, e] (the e-th largest softmax prob, selected via a
one-hot per-core input) and its weighted y_e partial; a per-token-block
ReduceScatter sums partials over cores, leaving each core with a 1/8 shard
of every block; the host concatenates shards.

Device data layouts (prepared host-side in kernel()):
    xt  (H, N)            x transposed (contraction dim H on partitions)
    wr  (128, KH*E)       wr[p, k*E+e] = Wr[e, k*128+p]
    w1  (IC, 128, H)      w1[ic, p, k*128+m] = W1[e][ic*128+m, k*128+p]  (= W1[e].T blocked)
    w2  (IC, 128, H)      w2[ic, p, h] = W2[e][h, ic*128+p]              (= W2[e].T strips)
    sel (128, E)          one-hot row (replicated over partitions) selecting rank e
    out (NBLK, NB//8, H)  per-core output: RS shard of every token block

PSUM budget: one unified 8-tag pool of (128, 512) fp32 tiles (1 bank each).
Routing logits, MM1 accumulators, and MM2 accumulators all rotate over the
same 8 tags, so total PSUM demand is exactly 8 banks.
"""

import numpy as np

import concourse.bacc as bacc
import concourse.bass as bass
import concourse.mybir as mybir
import concourse.tile as tile
from concourse.bass_utils import run_bass_kernel_spmd

AF = mybir.ActivationFunctionType
ALU = mybir.AluOpType
F32 = mybir.dt.float32

N_CORES = 8
B, S, H, I, E = 2, 2048, 1024, 4096, 8
N = B * S              # 4096 tokens
NB = 1024              # tokens per block
NBLK = N // NB         # 4 blocks
TPB = NB // 128        # 8 token tiles per block
TH = NB // 512         # 2 moving-operand token halves in MM1
KH = H // 128          # 8 contraction chunks for MM1
IC = I // 128          # 32 contraction chunks for MM2 / output chunks of MM1
HO = H // 512          # 2 moving-free slices of H in MM2


def build_nc(dt_mm=F32):
    nc = bacc.Bacc(None, num_devices=N_CORES)

    xt = nc.declare_dram_parameter("xt", [H, N], dt_mm, isOutput=False)
    wr = nc.declare_dram_parameter("wr", [128, KH * E], dt_mm, isOutput=False)
    w1 = nc.declare_dram_parameter("w1", [IC, 128, H], dt_mm, isOutput=False)
    w2 = nc.declare_dram_parameter("w2", [IC, 128, H], dt_mm, isOutput=False)
    sel = nc.declare_dram_parameter("sel", [128, E], F32, isOutput=False)
    out = nc.declare_dram_parameter("out", [NBLK, NB // N_CORES, H], F32, isOutput=True)

    xt_v = xt.ap().rearrange("(k p) n -> p k n", p=128)  # [p][k][n]

    with tile.TileContext(nc) as tc:
        with (
            tc.tile_pool(name="weights", bufs=1) as wpool,
            tc.tile_pool(name="io", bufs=1) as iopool,
            tc.tile_pool(name="w1s", bufs=4) as w1pool,
            tc.tile_pool(name="w2s", bufs=6) as w2pool,
            tc.tile_pool(name="outp", bufs=4) as opool,
            tc.tile_pool(name="hmidp", bufs=1) as hpool,
            tc.tile_pool(name="small", bufs=2) as smpool,
            tc.tile_pool(name="psum", bufs=1, space="PSUM") as pspool,
            tc.tile_pool(name="dram", bufs=2, space="DRAM") as dpool,
        ):
            def acc_tile(shape, name, slot):
                return pspool.tile(shape, F32, name=name, tag=f"a{slot % 8}",
                                   bufs=1, padded_shape=[128, 512])

            wr_sb = wpool.tile([128, KH * E], dt_mm, name="wr_sb", tag="wr_sb")
            nc.sync.dma_start(wr_sb[:], wr[:, :])
            sel_sb = wpool.tile([128, E], F32, name="sel_sb", tag="sel_sb")
            nc.sync.dma_start(sel_sb[:], sel[:, :])

            for b in range(NBLK):
                # ---- load xT block: (128, KH, NB)
                xtb = iopool.tile([128, KH, NB], dt_mm, name=f"xtb{b}", tag="xtb")
                nc.sync.dma_start(xtb[:], xt_v[:, :, b * NB:(b + 1) * NB])

                # ---- routing for the block's TPB token tiles
                wsel = smpool.tile([128, TPB], F32, name=f"wsel{b}", tag="wsel")
                for t in range(TPB):
                    tok = slice(t * 128, (t + 1) * 128)
                    lg = acc_tile([128, E], f"lg{b}_{t}", t)
                    for k in range(KH):
                        nc.tensor.matmul(
                            lg[:],
                            xtb[:, k, tok],
                            wr_sb[:, k * E:(k + 1) * E],
                            start=(k == 0),
                            stop=(k == KH - 1),
                        )
                    negm = smpool.tile([128, 1], F32, name=f"negm{b}_{t}", tag="negm")
                    nc.vector.tensor_reduce(negm[:], lg[:], axis=mybir.AxisListType.X,
                                            op=ALU.max, negate=True)
                    ev = smpool.tile([128, E], F32, name=f"ev{b}_{t}", tag="ev")
                    sumexp = smpool.tile([128, 1], F32, name=f"sumexp{b}_{t}", tag="sumexp")
                    nc.scalar.activation(ev[:], lg[:], AF.Exp, bias=negm[:],
                                         accum_out=sumexp[:])
                    rsum = smpool.tile([128, 1], F32, name=f"rsum{b}_{t}", tag="rsum")
                    nc.vector.reciprocal(rsum[:], sumexp[:])
                    # extract ranks 0..E-1 (descending) via max + mask-to-zero
                    wv = smpool.tile([128, E], F32, name=f"wv{b}_{t}", tag="wv")
                    for r in range(E):
                        nc.vector.tensor_reduce(wv[:, r:r + 1], ev[:],
                                                axis=mybir.AxisListType.X, op=ALU.max)
                        if r < E - 1:
                            # ev = (ev < max) * ev  : zero out the current max
                            nc.vector.scalar_tensor_tensor(
                                ev[:], ev[:], wv[:, r:r + 1], ev[:],
                                op0=ALU.is_lt, op1=ALU.mult)
                    # select this core's rank via one-hot, normalize by 1/sumexp
                    tmp8 = smpool.tile([128, E], F32, name=f"tmp8{b}_{t}", tag="tmp8")
                    nc.vector.tensor_mul(tmp8[:], wv[:], sel_sb[:])
                    nc.vector.tensor_reduce(wsel[:, t:t + 1], tmp8[:],
                                            axis=mybir.AxisListType.X, op=ALU.add)
                    nc.vector.tensor_mul(wsel[:, t:t + 1], wsel[:, t:t + 1], rsum[:])

                # ---- MM1 + GELU: hmid[ic] = gelu(W1_ic @ x_blk^T)  (128, NB) each
                hmid = hpool.tile([128, IC, NB], dt_mm, name=f"hmid{b}", tag="hmid")
                for ic in range(IC):
                    w1t = w1pool.tile([128, H], dt_mm, name=f"w1t{b}_{ic}", tag="w1t")
                    nc.sync.dma_start(w1t[:], w1[ic])
                    for th in range(TH):
                        tsl = slice(th * 512, (th + 1) * 512)
                        ps1 = acc_tile([128, 512], f"ps1{b}_{ic}_{th}", 2 * ic + th)
                        for k in range(KH):
                            nc.tensor.matmul(
                                ps1[:],
                                w1t[:, k * 128:(k + 1) * 128],
                                xtb[:, k, tsl],
                                start=(k == 0),
                                stop=(k == KH - 1),
                            )
                        nc.scalar.activation(hmid[:, ic, tsl], ps1[:], AF.Gelu)

                # ---- MM2: out[t] += hmid^T @ W2^T, weighted by wsel
                rs_in = dpool.tile([NB, H], F32, name=f"rsin{b}", tag="rsin")
                for h in range(HO):
                    hsl = slice(h * 512, (h + 1) * 512)
                    ps2 = [
                        acc_tile([128, 512], f"ps2_{b}_{h}_{t}", t)
                        for t in range(TPB)
                    ]
                    for ic in range(IC):
                        w2t = w2pool.tile([128, 512], dt_mm, name=f"w2t{b}_{h}_{ic}",
                                          tag="w2t")
                        nc.sync.dma_start(w2t[:], w2[ic, :, hsl])
                        for t in range(TPB):
                            nc.tensor.matmul(
                                ps2[t][:],
                                hmid[:, ic, t * 128:(t + 1) * 128],
                                w2t[:],
                                start=(ic == 0),
                                stop=(ic == IC - 1),
                            )
                    for t in range(TPB):
                        ot = opool.tile([128, 512], F32, name=f"ot{b}_{h}_{t}",
                                        tag="ot")
                        nc.vector.tensor_scalar_mul(ot[:], ps2[t][:],
                                                    wsel[:, t:t + 1])
                        nc.sync.dma_start(rs_in[t * 128:(t + 1) * 128, hsl], ot[:])

                # ---- combine over cores: ReduceScatter this block's partial
                rs_out = dpool.tile([NB // N_CORES, H], F32, name=f"rsout{b}",
                                    tag="rsout")
                nc.gpsimd.collective_compute(
                    "ReduceScatter",
                    ALU.add,
                    replica_groups=[list(range(N_CORES))],
                    ins=[rs_in.opt()],
                    outs=[rs_out.opt()],
                )
                nc.sync.dma_start(out[b], rs_out[:])

    nc.compile()
    return nc


def _prep_inputs(hidden_states, Wr, W1, W2, np_dt=np.float32):
    x = np.ascontiguousarray(
        np.asarray(hidden_states, dtype=np.float32).reshape(N, H))
    xT = np.ascontiguousarray(x.T).astype(np_dt, copy=False)
    Wr = np.asarray(Wr, dtype=np.float32)
    W1 = np.asarray(W1, dtype=np.float32)
    W2 = np.asarray(W2, dtype=np.float32)
    wrb = np.ascontiguousarray(
        Wr.T.reshape(KH, 128, E).transpose(1, 0, 2)).reshape(128, KH * E)
    wrb = wrb.astype(np_dt, copy=False)
    in_maps = []
    for e in range(N_CORES):
        w1b = np.ascontiguousarray(
            W1[e].reshape(IC, 128, KH, 128).transpose(0, 3, 2, 1)
        ).reshape(IC, 128, H).astype(np_dt, copy=False)
        w2b = np.ascontiguousarray(W2[e].T).reshape(IC, 128, H).astype(
            np_dt, copy=False)
        sel = np.zeros((128, E), np.float32)
        sel[:, e] = 1.0
        in_maps.append({"xt": xT, "wr": wrb, "w1": w1b, "w2": w2b, "sel": sel})
    return in_maps


_CACHE = {}


def run(inputs, trace=False, dt_mm=None, np_dt=np.float32):
    if dt_mm is None:
        dt_mm = mybir.dt.float32r
    key = str(dt_mm)
    if key not in _CACHE:
        _CACHE[key] = build_nc(dt_mm)
    nc = _CACHE[key]
    in_maps = _prep_inputs(inputs["hidden_states"], inputs["Wr"], inputs["W1"],
                           inputs["W2"], np_dt=np_dt)
    res = run_bass_kernel_spmd(nc, in_maps, list(range(N_CORES)), trace=trace)
    shards = np.stack([np.asarray(res.results[r]["out"]) for r in range(N_CORES)],
                      axis=1)  # (NBLK, cores, NB//cores, H)
    full = shards.reshape(B, S, H).astype(np.float32, copy=False)
    return full, res


def kernel(**inputs) -> np.ndarray:
    full, _ = run(inputs, trace=False)
    return full


# revision 13
# speedup vs baseline: 1.0832x; 1.0302x over previous
"""MoE MLP (top-k == E) Trainium2 Bass kernel, expert-parallel over 8 NeuronCores.

Reference computation (per token x, experts e = 0..7):
    logits = x @ Wr.T                       (N, E)
    probs  = softmax(logits)                (N, E)
    w      = sort_desc(probs)               (N, E)   # rank-sorted, NOT matched to expert id
    hmid_e = gelu_erf(x @ W1[e].T)          (N, I)
    y_e    = hmid_e @ W2[e].T               (N, H)
    out    = sum_e w[:, e] * y_e            (N, H)

Sharding: core e owns expert e (W1[e], W2[e]); tokens + router replicated.
Each core computes w[:, e] (the e-th largest softmax prob, selected via a
one-hot per-core input) and its weighted y_e partial; a per-token-block
ReduceScatter sums partials over cores, leaving each core with a 1/8 shard
of every block; the host concatenates shards.

Device data layouts (prepared host-side in kernel()):
    xt  (H, N)            x transposed (contraction dim H on partitions)
    wr  (128, KH*E)       wr[p, k*E+e] = Wr[e, k*128+p]
    w1  (IC, 128, H)      w1[ic, p, k*128+m] = W1[e][ic*128+m, k*128+p]  (= W1[e].T blocked)
    w2  (IC, 128, H)      w2[ic, p, h] = W2[e][h, ic*128+p]              (= W2[e].T strips)
    sel (128, E)          one-hot row (replicated over partitions) selecting rank e
    out (NBLK, NB//8, H)  per-core output: RS shard of every token block

PSUM budget: one unified 8-tag pool of (128, 512) fp32 tiles (1 bank each).
Routing logits, MM1 accumulators, and MM2 accumulators all rotate over the
same 8 tags, so total PSUM demand is exactly 8 banks.
"""

import numpy as np

import concourse.bacc as bacc
import concourse.bass as bass
import concourse.mybir as mybir
import concourse.tile as tile
from concourse.bass_utils import run_bass_kernel_spmd

AF = mybir.ActivationFunctionType
ALU = mybir.AluOpType
F32 = mybir.dt.float32

N_CORES = 8
B, S, H, I, E = 2, 2048, 1024, 4096, 8
N = B * S              # 4096 tokens
NB = 1024              # tokens per block
NBLK = N // NB         # 4 blocks
TPB = NB // 128        # 8 token tiles per block
TH = NB // 512         # 2 moving-operand token halves in MM1
KH = H // 128          # 8 contraction chunks for MM1
IC = I // 128          # 32 contraction chunks for MM2 / output chunks of MM1
HO = H // 512          # 2 moving-free slices of H in MM2


def build_nc(dt_mm=F32):
    nc = bacc.Bacc(None, num_devices=N_CORES)

    xt = nc.declare_dram_parameter("xt", [H, N], dt_mm, isOutput=False)
    wr = nc.declare_dram_parameter("wr", [128, KH * E], dt_mm, isOutput=False)
    w1 = nc.declare_dram_parameter("w1", [IC, 128, H], dt_mm, isOutput=False)
    w2 = nc.declare_dram_parameter("w2", [IC, 128, H], dt_mm, isOutput=False)
    sel = nc.declare_dram_parameter("sel", [128, E], F32, isOutput=False)
    out = nc.declare_dram_parameter("out", [NBLK, HO, NB // N_CORES, 512], F32,
                                    isOutput=True)

    xt_v = xt.ap().rearrange("(k p) n -> p k n", p=128)  # [p][k][n]

    with tile.TileContext(nc) as tc:
        with (
            tc.tile_pool(name="weights", bufs=1) as wpool,
            tc.tile_pool(name="io", bufs=1) as iopool,
            tc.tile_pool(name="w1s", bufs=4) as w1pool,
            tc.tile_pool(name="w2s", bufs=6) as w2pool,
            tc.tile_pool(name="outp", bufs=4) as opool,
            tc.tile_pool(name="hmidp", bufs=1) as hpool,
            tc.tile_pool(name="small", bufs=2) as smpool,
            tc.tile_pool(name="psum", bufs=1, space="PSUM") as pspool,
            tc.tile_pool(name="dram", bufs=2, space="DRAM") as dpool,
        ):
            def acc_tile(shape, name, slot):
                return pspool.tile(shape, F32, name=name, tag=f"a{slot % 8}",
                                   bufs=1, padded_shape=[128, 512])

            wr_sb = wpool.tile([128, KH * E], dt_mm, name="wr_sb", tag="wr_sb")
            nc.sync.dma_start(wr_sb[:], wr[:, :])
            sel_sb = wpool.tile([128, E], F32, name="sel_sb", tag="sel_sb")
            nc.sync.dma_start(sel_sb[:], sel[:, :])

            def load_xtb(b):
                # per-k split DMAs on the ACT queue: issue position is
                # independent of the sync queue's weight stream, and routing
                # matmul k can start as soon as chunk k lands
                xtb = iopool.tile([128, KH, NB], dt_mm, name=f"xtb{b}", tag="xtb")
                for k in range(KH):
                    nc.scalar.dma_start(xtb[:, k, :],
                                        xt_v[:, k, b * NB:(b + 1) * NB])
                return xtb

            xtb_next = load_xtb(0)

            for b in range(NBLK):
                xtb = xtb_next

                # ---- routing for the block's TPB token tiles
                wsel = smpool.tile([128, TPB], F32, name=f"wsel{b}", tag="wsel")
                for t in range(TPB):
                    tok = slice(t * 128, (t + 1) * 128)
                    lg = acc_tile([128, E], f"lg{b}_{t}", t)
                    for k in range(KH):
                        nc.tensor.matmul(
                            lg[:],
                            xtb[:, k, tok],
                            wr_sb[:, k * E:(k + 1) * E],
                            start=(k == 0),
                            stop=(k == KH - 1),
                        )
                    negm = smpool.tile([128, 1], F32, name=f"negm{b}_{t}", tag="negm")
                    nc.vector.tensor_reduce(negm[:], lg[:], axis=mybir.AxisListType.X,
                                            op=ALU.max, negate=True)
                    ev = smpool.tile([128, E], F32, name=f"ev{b}_{t}", tag="ev")
                    sumexp = smpool.tile([128, 1], F32, name=f"sumexp{b}_{t}", tag="sumexp")
                    nc.scalar.activation(ev[:], lg[:], AF.Exp, bias=negm[:],
                                         accum_out=sumexp[:])
                    rsum = smpool.tile([128, 1], F32, name=f"rsum{b}_{t}", tag="rsum")
                    nc.vector.reciprocal(rsum[:], sumexp[:])
                    # extract ranks 0..E-1 (descending) via max + mask-to-zero
                    wv = smpool.tile([128, E], F32, name=f"wv{b}_{t}", tag="wv")
                    for r in range(E):
                        nc.vector.tensor_reduce(wv[:, r:r + 1], ev[:],
                                                axis=mybir.AxisListType.X, op=ALU.max)
                        if r < E - 1:
                            # ev = (ev < max) * ev  : zero out the current max
                            nc.vector.scalar_tensor_tensor(
                                ev[:], ev[:], wv[:, r:r + 1], ev[:],
                                op0=ALU.is_lt, op1=ALU.mult)
                    # select this core's rank via one-hot, normalize by 1/sumexp
                    tmp8 = smpool.tile([128, E], F32, name=f"tmp8{b}_{t}", tag="tmp8")
                    nc.vector.tensor_mul(tmp8[:], wv[:], sel_sb[:])
                    nc.vector.tensor_reduce(wsel[:, t:t + 1], tmp8[:],
                                            axis=mybir.AxisListType.X, op=ALU.add)
                    nc.vector.tensor_mul(wsel[:, t:t + 1], wsel[:, t:t + 1], rsum[:])

                # ---- MM1 + GELU: hmid[ic] = gelu(W1_ic @ x_blk^T)  (128, NB) each
                hmid = hpool.tile([128, IC, NB], dt_mm, name=f"hmid{b}", tag="hmid")
                for ic in range(IC):
                    w1t = w1pool.tile([128, H], dt_mm, name=f"w1t{b}_{ic}", tag="w1t")
                    nc.sync.dma_start(w1t[:], w1[ic])
                    for th in range(TH):
                        tsl = slice(th * 512, (th + 1) * 512)
                        ps1 = acc_tile([128, 512], f"ps1{b}_{ic}_{th}", 2 * ic + th)
                        for k in range(KH):
                            nc.tensor.matmul(
                                ps1[:],
                                w1t[:, k * 128:(k + 1) * 128],
                                xtb[:, k, tsl],
                                start=(k == 0),
                                stop=(k == KH - 1),
                            )
                        nc.scalar.activation(hmid[:, ic, tsl], ps1[:], AF.Gelu)

                # prefetch next block's tokens while MM2 runs
                if b + 1 < NBLK:
                    xtb_next = load_xtb(b + 1)

                # ---- MM2: out[t] += hmid^T @ W2^T, weighted by wsel
                for h in range(HO):
                    hsl = slice(h * 512, (h + 1) * 512)
                    rs_in = dpool.tile([NB, 512], F32, name=f"rsin{b}_{h}",
                                       tag="rsin")
                    ps2 = [
                        acc_tile([128, 512], f"ps2_{b}_{h}_{t}", t)
                        for t in range(TPB)
                    ]
                    for ic in range(IC):
                        w2t = w2pool.tile([128, 512], dt_mm, name=f"w2t{b}_{h}_{ic}",
                                          tag="w2t")
                        nc.sync.dma_start(w2t[:], w2[ic, :, hsl])
                        for t in range(TPB):
                            nc.tensor.matmul(
                                ps2[t][:],
                                hmid[:, ic, t * 128:(t + 1) * 128],
                                w2t[:],
                                start=(ic == 0),
                                stop=(ic == IC - 1),
                            )
                    for t in range(TPB):
                        ot = opool.tile([128, 512], F32, name=f"ot{b}_{h}_{t}",
                                        tag="ot")
                        nc.vector.tensor_scalar_mul(ot[:], ps2[t][:],
                                                    wsel[:, t:t + 1])
                        nc.sync.dma_start(rs_in[t * 128:(t + 1) * 128, :], ot[:])

                    # ---- combine over cores: ReduceScatter this h-half
                    rs_out = dpool.tile([NB // N_CORES, 512], F32,
                                        name=f"rsout{b}_{h}", tag="rsout")
                    nc.gpsimd.collective_compute(
                        "ReduceScatter",
                        ALU.add,
                        replica_groups=[list(range(N_CORES))],
                        ins=[rs_in.opt()],
                        outs=[rs_out.opt()],
                    )
                    nc.sync.dma_start(out[b, h], rs_out[:])

    nc.compile()
    return nc


def _prep_inputs(hidden_states, Wr, W1, W2, np_dt=np.float32):
    x = np.ascontiguousarray(
        np.asarray(hidden_states, dtype=np.float32).reshape(N, H))
    xT = np.ascontiguousarray(x.T).astype(np_dt, copy=False)
    Wr = np.asarray(Wr, dtype=np.float32)
    W1 = np.asarray(W1, dtype=np.float32)
    W2 = np.asarray(W2, dtype=np.float32)
    wrb = np.ascontiguousarray(
        Wr.T.reshape(KH, 128, E).transpose(1, 0, 2)).reshape(128, KH * E)
    wrb = wrb.astype(np_dt, copy=False)
    in_maps = []
    for e in range(N_CORES):
        w1b = np.ascontiguousarray(
            W1[e].reshape(IC, 128, KH, 128).transpose(0, 3, 2, 1)
        ).reshape(IC, 128, H).astype(np_dt, copy=False)
        w2b = np.ascontiguousarray(W2[e].T).reshape(IC, 128, H).astype(
            np_dt, copy=False)
        sel = np.zeros((128, E), np.float32)
        sel[:, e] = 1.0
        in_maps.append({"xt": xT, "wr": wrb, "w1": w1b, "w2": w2b, "sel": sel})
    return in_maps


_CACHE = {}


def run(inputs, trace=False, dt_mm=None, np_dt=np.float32):
    if dt_mm is None:
        dt_mm = mybir.dt.float32r
    key = str(dt_mm)
    if key not in _CACHE:
        _CACHE[key] = build_nc(dt_mm)
    nc = _CACHE[key]
    in_maps = _prep_inputs(inputs["hidden_states"], inputs["Wr"], inputs["W1"],
                           inputs["W2"], np_dt=np_dt)
    res = run_bass_kernel_spmd(nc, in_maps, list(range(N_CORES)), trace=trace)
    # per-core out: (NBLK, HO, NB//cores, 512)
    shards = np.stack([np.asarray(res.results[r]["out"]) for r in range(N_CORES)],
                      axis=1)  # (NBLK, cores, HO, NB//cores, 512)
    full = shards.transpose(0, 1, 3, 2, 4).reshape(B, S, H)
    return np.ascontiguousarray(full, dtype=np.float32), res


def kernel(**inputs) -> np.ndarray:
    full, _ = run(inputs, trace=False)
    return full
